# revision 2
# baseline (speedup 1.0000x reference)
"""GCN layer kernel for Trainium2, 8-core row-parallel.

Computes out = (adj * mask + I) @ (x @ W^T) for N=8192, C_in=C_out=128.

Sharding: adj/mask row-blocks of 1024 across 8 cores; x, W replicated.
v2 (natural k-block layout; replaces the 8-way column-interleaved
j-permutation of v1):
  - adj+mask stream on the SP HWDGE ring in 1MB half-chunk dma_starts
    (4KB descriptors); triggers prefetched PREF chunks ahead; out writes
    interleave on the same ring.
  - product adj*mask -> separate bf16 tile (adj AND mask slots free at
    the mul); h bf16; accumulation f32 in PSUM.
  - x loaded in NATURAL 128-row blocks ("(j p) c" AP, 512B descriptors)
    on the scalar HWDGE ring, so h tiles are in natural row order and
    each k-block pipe (PE transpose of a contiguous 128-col slice of
    prod -> PSUM->SBUF copy -> 512-wide bf16 matmul) depends only on
    the mul HALF that covers it: k-blocks 0-3 run off mul-half0 while
    half1 still streams.  v1's j-interleave needed the full chunk mul
    before ANY j-pipe, which serialized the endgame (~25us of DMA
    dribble + drain at the tail of the v1 trace).
  - PSUM->SBUF at-copies alternate ACT (even k-block) / DVE (odd), so
    neither engine's in-order stream is the drain bottleneck; DVE's
    stream stays muls-first per pair.
  - last chunk streams in 4 quarter-DMAs with 4 quarter-muls so the
    final kb-pipes start as early as possible; finalize(blk0) overlaps
    the last chunk, finalize(blk1) is the only post-stream work.
  - finalize uses a 4-rows-per-partition permuted layout so the out
    write has 2KB descriptors; x_own/ho use the same permutation.
"""

import numpy as np
from contextlib import ExitStack

from concourse import bass, bacc, tile, mybir
from concourse import masks
from concourse.bass_utils import run_bass_kernel_spmd

N = 8192
C = 128
NCORES = 8
R = N // NCORES          # 1024 rows per core
M_BLK = 512              # psum accumulation block (free dim of main matmul)
NBLK = R // M_BLK        # 2 m-blocks per core
S = M_BLK // 128         # 4 slabs of 128 rows per m-block
KQ = 1024                # k-chunk width per DMA iteration
NQ = N // KQ             # 8 k-chunks per m-block
XJ = KQ // 128           # 8 natural 128-row k-blocks per chunk
JF = 4                   # finalize: rows per partition (out descriptor = JF*512B)
PREF = 3                 # chunks of DMA-trigger prefetch ahead of compute

F32 = mybir.dt.float32
F32R = mybir.dt.float32r
BF16 = mybir.dt.bfloat16


def build_program():
    nc = bacc.Bacc("TRN2", target_bir_lowering=False, debug=False, num_devices=NCORES)

    adj_d = nc.dram_tensor("adj", [R, N], F32, kind="ExternalInput").ap()
    mask_d = nc.dram_tensor("mask", [R, N], F32, kind="ExternalInput").ap()
    x_d = nc.dram_tensor("x", [N, C], F32, kind="ExternalInput").ap()
    xo_d = nc.dram_tensor("x_own", [R, C], F32, kind="ExternalInput").ap()
    w_d = nc.dram_tensor("w", [C, C], F32, kind="ExternalInput").ap()
    out_d = nc.dram_tensor("out", [R, C], F32, kind="ExternalOutput").ap()

    with tile.TileContext(nc) as tc, ExitStack() as ctx:
        const_pool = ctx.enter_context(tc.tile_pool(name="const", bufs=1))
        xg_pool = ctx.enter_context(tc.tile_pool(name="xg", bufs=2))
        xt_pool = ctx.enter_context(tc.tile_pool(name="xt", bufs=3))
        h_pool = ctx.enter_context(tc.tile_pool(name="h", bufs=1))
        adj_pool = ctx.enter_context(tc.tile_pool(name="adj", bufs=4))
        mask_pool = ctx.enter_context(tc.tile_pool(name="mask", bufs=4))
        prod_pool = ctx.enter_context(tc.tile_pool(name="prod", bufs=3))
        at_pool = ctx.enter_context(tc.tile_pool(name="at", bufs=6))
        fin_pool = ctx.enter_context(tc.tile_pool(name="fin", bufs=4))
        psum_acc = ctx.enter_context(tc.tile_pool(name="pacc", bufs=2, space="PSUM"))
        psum_tr = ctx.enter_context(tc.tile_pool(name="ptr", bufs=3, space="PSUM"))
        psum_misc = ctx.enter_context(tc.tile_pool(name="pmisc", bufs=2, space="PSUM"))
        psum_fin = ctx.enter_context(tc.tile_pool(name="pfin", bufs=1, space="PSUM"))

        ident = const_pool.tile([128, 128], F32)
        masks.make_identity(nc, ident[:])
        identb = const_pool.tile([128, 128], BF16)
        nc.vector.tensor_copy(identb[:], ident[:])

        # ---- weight: W^T in bf16, loaded on the gpsimd queue ----
        w_sb = const_pool.tile([128, C], F32)
        nc.gpsimd.dma_start(out=w_sb[:], in_=w_d[:, :])
        psum_wt = psum_misc.tile([128, 128], F32, tag="pm")
        nc.tensor.transpose(psum_wt[:], w_sb[:], ident[:])
        wtr_sb = const_pool.tile([128, C], BF16)
        nc.vector.tensor_copy(wtr_sb[:], psum_wt[:])

        # ---- x loads: 8 groups of 1024 rows, NATURAL 128-row blocks
        # (row = j*128 + p), 512B descriptors, on the scalar HWDGE ring
        xg_tiles = []
        for g in range(NQ):
            xg = xg_pool.tile([128, XJ, C], F32, tag="xg")
            nc.scalar.dma_start(
                out=xg[:],
                in_=x_d[g * KQ : (g + 1) * KQ, :].rearrange(
                    "(j p) c -> p j c", p=128, j=XJ
                ),
            )
            xg_tiles.append(xg)
        # x_own in the finalize permutation: row blk*512 + p*JF + j
        xo_sb = const_pool.tile([128, NBLK * JF, C], F32)
        for b in range(NBLK):
            nc.scalar.dma_start(
                out=xo_sb[:, b * JF : (b + 1) * JF, :],
                in_=xo_d[b * M_BLK : (b + 1) * M_BLK, :].rearrange(
                    "(p j) c -> p j c", p=128, j=JF
                ),
            )

        # h tile (g, b) holds rows g*1024 + b*128 + p (natural order)
        h_sb = h_pool.tile([128, NQ * XJ, C], BF16)
        ho_sb = const_pool.tile([128, NBLK * JF, C], F32)

        def h_tile_pipe(src_view, dst_view):
            psum_xt = psum_misc.tile([128, 128], F32, tag="pm")
            nc.tensor.transpose(psum_xt[:], src_view, ident[:])
            xt_sb = xt_pool.tile([128, 128], BF16)
            nc.scalar.copy(xt_sb[:], psum_xt[:])  # f32 -> bf16 rounds here
            psum_h = psum_misc.tile([128, 128], F32, tag="pm")
            nc.tensor.matmul(psum_h[:], xt_sb[:], wtr_sb[:], start=True, stop=True)
            nc.scalar.copy(dst_view, psum_h[:])

        def phase0_group(g):
            for b in range(XJ):
                h_tile_pipe(xg_tiles[g][:, b, :], h_sb[:, g * XJ + b, :])

        def phase0_own():
            # self-loop h in the finalize permutation (row blk*512 + JF*p + j)
            for j in range(NBLK * JF):
                h_tile_pipe(xo_sb[:, j, :], ho_sb[:, j, :])

        # ---- main loop ----
        def emit_triggers(blk, q, parts):
            r0 = blk * M_BLK
            k0 = q * KQ
            adj_t = adj_pool.tile([128, S, KQ], F32, tag="adj")
            mask_t = mask_pool.tile([128, S, KQ], F32, tag="mask")
            pw = KQ // parts
            for hh in range(parts):
                sl = slice(hh * pw, (hh + 1) * pw)
                nc.sync.dma_start(
                    out=adj_t[:, :, sl],
                    in_=adj_d[r0 : r0 + M_BLK, k0 + hh * pw : k0 + (hh + 1) * pw]
                    .rearrange("(s p) k -> p s k", p=128),
                )
                nc.sync.dma_start(
                    out=mask_t[:, :, sl],
                    in_=mask_d[r0 : r0 + M_BLK, k0 + hh * pw : k0 + (hh + 1) * pw]
                    .rearrange("(s p) k -> p s k", p=128),
                )
            return adj_t, mask_t

        def emit_muls(adj_t, mask_t, parts=2):
            # separate bf16 product tile: adj AND mask slots free at the mul
            prod_t = prod_pool.tile([128, S, KQ], BF16, tag="prod")
            pw = KQ // parts
            for m in range(parts):
                sl = slice(m * pw, (m + 1) * pw)
                nc.vector.tensor_mul(
                    prod_t[:, :, sl], adj_t[:, :, sl], mask_t[:, :, sl]
                )
            return prod_t

        def emit_kbpipes(pacc, q, prod_t):
            for b in range(XJ):
                psum_at = psum_tr.tile([128, M_BLK], BF16)
                for s in range(S):
                    # contiguous 128-col k-block of slab s; its transpose
                    # has partition f <-> k = q*1024 + b*128 + f, matching
                    # natural h tile (q, b)
                    nc.tensor.transpose(
                        psum_at[:, s * 128 : (s + 1) * 128],
                        prod_t[:, s, b * 128 : (b + 1) * 128],
                        identb[:],
                    )
                at_sb = at_pool.tile([128, M_BLK], BF16)
                if b % 2 == 0:
                    nc.scalar.copy(at_sb[:], psum_at[:])
                else:
                    nc.vector.tensor_copy(at_sb[:], psum_at[:])
                kg = q * XJ + b
                nc.tensor.matmul(
                    pacc[:],
                    h_sb[:, kg, :],
                    at_sb[:],
                    start=(kg == 0),
                    stop=(kg == NQ * XJ - 1),
                )

        def finalize(blk, pacc):
            # out rows blk*512 + JF*p + j; 2KB out descriptors
            psum_nat = psum_fin.tile([128, JF, C], F32)
            pacc_j = pacc[:].rearrange("p (m j) -> p j m", j=JF)
            for j in range(JF):
                otj = fin_pool.tile([128, 128], F32, tag="fin_t")
                nc.vector.tensor_copy(otj[:], pacc_j[:, j, :])
                nc.tensor.transpose(psum_nat[:, j, :], otj[:], ident[:])
            out_sb = fin_pool.tile([128, JF, C], F32, tag="fin_o")
            nc.vector.tensor_add(
                out_sb[:],
                psum_nat[:],
                ho_sb[:, blk * JF : (blk + 1) * JF, :],
            )
            r0 = blk * M_BLK
            nc.sync.dma_start(
                out=out_d[r0 : r0 + M_BLK, :].rearrange("(p j) c -> p j c", p=128),
                in_=out_sb[:],
            )

        # q-major pairs (blk0,q),(blk1,q): per pair emit BOTH chunks' muls
        # first (DVE stays muls-first), then phase-0 group q (PE + ACT
        # copies), then the kb-pipes.  Both paccs accumulate simultaneously.
        chunks = [(blk, q) for q in range(NQ) for blk in range(NBLK)]
        last = len(chunks) - 1
        paccs = {
            blk: psum_acc.tile([128, M_BLK], F32, name="pacc")
            for blk in range(NBLK)
        }
        trigs = {}
        for k in range(PREF):
            trigs[k] = emit_triggers(*chunks[k], parts=2)
        for pq in range(NQ):
            i0 = 2 * pq
            for i in (i0, i0 + 1):
                if i + PREF <= last:
                    trigs[i + PREF] = emit_triggers(
                        *chunks[i + PREF], parts=(4 if i + PREF == last else 2)
                    )
            prods = {}
            for i in (i0, i0 + 1):
                prods[i] = emit_muls(*trigs.pop(i), parts=(4 if i == last else 2))
            phase0_group(pq)
            if pq == 4:
                phase0_own()
            for i in (i0, i0 + 1):
                blk, q = chunks[i]
                if i == last:
                    # blk0's accumulation finished one chunk ago; its
                    # finalize runs concurrent with the last chunk
                    finalize(0, paccs[0])
                emit_kbpipes(paccs[blk], q, prods.pop(i))
        finalize(NBLK - 1, paccs[NBLK - 1])

    nc.compile()
    return nc


_NC_CACHE = None


def _get_nc():
    global _NC_CACHE
    if _NC_CACHE is None:
        _NC_CACHE = build_program()
    return _NC_CACHE


def kernel(x, adj, mask, W):
    x = np.ascontiguousarray(x, dtype=np.float32)
    adj = np.ascontiguousarray(adj, dtype=np.float32)
    mask = np.ascontiguousarray(mask, dtype=np.float32)
    W = np.ascontiguousarray(W, dtype=np.float32)

    nc = _get_nc()
    in_maps = []
    for i in range(NCORES):
        r0 = i * R
        in_maps.append(
            {
                "adj": adj[r0 : r0 + R],
                "mask": mask[r0 : r0 + R],
                "x": x,
                "x_own": x[r0 : r0 + R],
                "w": W,
            }
        )
    res = run_bass_kernel_spmd(nc, in_maps, list(range(NCORES)))
    return np.concatenate([res.results[i]["out"] for i in range(NCORES)], axis=0)
